# revision 30
# baseline (speedup 1.0000x reference)
"""BinaryDense kernel for Trainium2: out = sign(x) @ sign(w).

x: [8192, 2048] f32, w: [2048, 2048] f32 -> out: [8192, 2048] f32.

Strategy: data-parallel shard of the batch dim across 8 NeuronCores
(1024 rows each, w replicated). The host ships only the HIGH BYTE of
each f32 (a pure byte-plane slice -- sign bit + 7 exponent bits, which
fully determines the sign), so input DMA is 6MB/core (x 2MB + w 4MB).
Host pre-tiles layouts so every DMA granule moves contiguous >=512B
lines per partition and every binarize op is fully contiguous:
  x_dram [128p, 2g, 2e, 16ks, 256m]  (d = ks*128 + p,
                                      m = g*512 + e*256 + m'')
  w_dram [128p, 4q, 16ks, 512u]      (u = q*512 + u')
Per core:
  - input DMAs are the first instructions on both HWDGE rings (x on
    sync, w on scalar), so data flows during the engine prologue;
  - PE warm-up dummy matmuls with NO dependencies (they read an
    uninitialized scratch tile; results are discarded) start the HAM
    clock ramp at the earliest point and bridge until real operands
    arrive, so the clock never re-gates;
  - binarized operands live in DEPENDENCY-EXACT tiles (the Tile
    framework tracks deps per tile): one [128,2,256] x-tile per
    (h-step, m-group, engine class) and one [128,2,512] w-tile per
    (pass, h-step), each written by exactly ONE binarize op, so every
    matmul waits on precisely the data it consumes;
  - binarize from the u8 high byte ((v < 128) <=> x >= 0), split
    across both elementwise engines by output-column class (the per-j
    evict scale absorbs the class difference):
      all w, and x e=1 cols (j2-3/j6-7) -> +-0.5 fp8e4 on DVE
      x e=0 cols (j0-1/j4-5)            -> +-1  fp8e4 on ACT
  - fp8 DoubleRow matmuls (K=256/instr, N=512): stationary = x slice,
    moving = w tile. Pass 0 runs h-major rows over m-group 0 then 1
    (8 PSUM banks = 8 concurrent chains), woven with the arrival
    stream; passes 1-3 run 4-wide j-groups with w prefetched a pass
    ahead (q2/q3 as single 1MB DMAs) and evicts of the previous group
    on the first rows. The final group runs as two ping-pong chain
    pairs with inline evict+store so the post-stream tail is one
    evict pair + two 128KB stores;
  - outputs in per-(j,q) fp16 tiles (integers <= 2048, exact); ring
    policy: ALL input DMAs issue on the sync ring (its queue is
    otherwise empty; the scalar ring's issue queue is shared with ACT
    compute and head-of-line blocks), mid-kernel stores go on the
    idle gpsimd SWDGE ring, and only the final two stores use the
    (by then empty) sync+scalar rings.

All arithmetic exact; host fp16->f32 widening exact.
"""

import sys

if "/opt/trn_rl_repo" not in sys.path:
    sys.path.insert(0, "/opt/trn_rl_repo")

import numpy as np

B_FULL, D_IN, UNITS = 8192, 2048, 2048
N_CORES = 8
B_CORE = B_FULL // N_CORES  # 1024
P = 128
KT = D_IN // P              # 16 k-subtiles
NQ = 4                      # n passes of 512 columns
NH = 8                      # DR h-steps per pass (2 k-subtiles each)
MT = B_CORE // P            # 8 m-tiles
N_DUMMY = 10                # PE warm-up matmuls


def build_kernel():
    from concourse import bacc
    import concourse.mybir as mybir
    import concourse.tile as tile

    f32 = mybir.dt.float32
    f16 = mybir.dt.float16
    f8 = mybir.dt.float8e4
    u8 = mybir.dt.uint8

    LT = mybir.AluOpType.is_lt
    SUB = mybir.AluOpType.subtract
    COPY = mybir.ActivationFunctionType.Copy
    SIGN = mybir.ActivationFunctionType.Sign
    DR = mybir.MatmulPerfMode.DoubleRow

    # per-j evict scale: chain sum = alpha * out, w always +-0.5 (DVE)
    #   x on ACT (+-1, e=0) -> alpha 0.5 -> scale 2 : j0-1, j4-5
    #   x on DVE (+-0.5, e=1) -> alpha 0.25 -> scale 4 : j2-3, j6-7
    EV_SCALE_J = (2.0, 2.0, 4.0, 4.0, 2.0, 2.0, 4.0, 4.0)

    nc = bacc.Bacc("TRN2", target_bir_lowering=False)
    # const AP for the Sign-activation bias (127.5). The memset runs on
    # gpsimd immediately after the framework's init barrier; its first
    # reader (ACT Sign) is gated on input-DMA arrival, so no extra
    # barrier is needed.
    _bt = nc.alloc_sbuf_tensor("const-f32-127p5", [P, 1], f32)
    nc.gpsimd.memset(_bt.ap(), 127.5)
    nc.const_aps.aps[(f32, 127.5)] = _bt.ap()
    # scratch for the PE warm-up dummies: deliberately UNINITIALIZED.
    # The dummies only exist to keep the PE clock ramping; their
    # results go to a psum bank that is overwritten (start=True).
    _scratch = nc.alloc_sbuf_tensor("warm-scratch", [P, 2, 512], f8)

    x_d = nc.dram_tensor("xhi", [P, 2, 2, KT, 256], u8,
                         kind="ExternalInput")
    w_d = nc.dram_tensor("whi", [P, NQ, KT, 512], u8, kind="ExternalInput")
    o_d = nc.dram_tensor("out", [B_CORE, UNITS], f16, kind="ExternalOutput")

    o_ap = o_d[:].rearrange("(j p) u -> j p u", p=P)  # [MT, 128, U]

    with tile.TileContext(nc) as tc, \
         tc.tile_pool(name="wstage", bufs=10) as wstage, \
         tc.tile_pool(name="wbig", bufs=2) as wbig, \
         tc.tile_pool(name="xstage", bufs=9) as xstage, \
         tc.tile_pool(name="resident", bufs=1) as resident, \
         tc.tile_pool(name="mpsum", bufs=8, space="PSUM") as mpsum:

        # dependency-exact resident tiles, one writer each:
        # xh[h][g][e]: ksubs 2h..2h+1, m-cols g*512+e*256 .. +255
        xh = [[[resident.tile([P, 2, 256], f8, name=f"xh_{h}_{g}_{e}")
                for e in range(2)] for g in range(2)] for h in range(NH)]
        # w8h[q][h]: pass q, ksubs 2h..2h+1
        w8h = [[resident.tile([P, 2, 512], f8, name=f"w8_{q}_{h}")
                for h in range(NH)] for q in range(NQ)]
        # per-(j, q) output tiles: each written by ONE evict and read by
        # ONE store, so stores stream as soon as their chain evicts
        ost = [[resident.tile([P, 512], f16, name=f"ost_{j}_{q}")
                for q in range(NQ)] for j in range(MT)]

        # ---- head DMAs: first instructions on both HWDGE rings ----
        # t=0 granules split in k-halves; x halves on sync, w halves on
        # scalar, so the packet round-robin delivers the h=0 operand
        # pair (xsa+wsa) first, then the h=1 pair.
        xsa = xstage.tile([P, 2, 2, 256], u8, tag="xs", name="xs_0a")
        nc.sync.dma_start(xsa, x_d[:][:, 0, :, 0:2, :])
        wsa = wstage.tile([P, 2, 512], u8, tag="ws", name="ws_0a")
        nc.scalar.dma_start(wsa, w_d[:][:, 0, 0:2, :])
        xsb = xstage.tile([P, 2, 2, 256], u8, tag="xs", name="xs_0b")
        nc.sync.dma_start(xsb, x_d[:][:, 0, :, 2:4, :])
        wsb = wstage.tile([P, 2, 512], u8, tag="ws", name="ws_0b")
        nc.scalar.dma_start(wsb, w_d[:][:, 0, 2:4, :])

        # ---- PE warm-up: dummy matmuls with no dependencies ----
        ps_warm = mpsum.tile([P, 512], f32, tag="ps", name="ps_warm")
        for _ in range(N_DUMMY):
            nc.tensor.matmul(ps_warm, lhsT=_scratch.ap()[:, :, 0:P],
                             rhs=_scratch.ap(),
                             start=True, stop=True, perf_mode=DR)

        def dma_x(t, g, ring):
            xs = xstage.tile([P, 2, 4, 256], u8, tag="xs",
                             name=f"xs_{t}_{g}")
            ring.dma_start(xs, x_d[:][:, g, :, 4 * t:4 * t + 4, :])
            return xs

        def dma_w(q, hp, ring):
            ws = wstage.tile([P, 4, 512], u8, tag="ws", name=f"ws_{q}_{hp}")
            ring.dma_start(ws, w_d[:][:, q, 4 * hp:4 * hp + 4, :])
            return ws

        def dma_w_pass(q, ring):
            # whole 1MB pass prefetch in one DMA (8KB/partition lines)
            ws = wbig.tile([P, KT, 512], u8, tag="wb", name=f"wsbig_{q}")
            ring.dma_start(ws, w_d[:][:, q, :, :])
            return ws

        def bin_x(t, g, xs):
            # one contiguous op per (h, e): exact dep for its mms
            for c in (0, 2):
                h = 2 * t + c // 2
                nc.scalar.activation(xh[h][g][0], xs[:, 0, c:c + 2, :],
                                     SIGN, bias=127.5, scale=-1.0)
                nc.vector.tensor_scalar(xh[h][g][1], xs[:, 1, c:c + 2, :],
                                        128.0, 0.5, LT, SUB)

        def bin_w(q, hp, ws):
            for c in (0, 2):
                nc.vector.tensor_scalar(w8h[q][2 * hp + c // 2],
                                        ws[:, c:c + 2, :], 128.0, 0.5,
                                        LT, SUB)

        def bin_w_big(q, h, ws):
            nc.vector.tensor_scalar(w8h[q][h], ws[:, 2 * h:2 * h + 2, :],
                                    128.0, 0.5, LT, SUB)

        psum_tiles = {}

        def mm(q, j, h):
            g = j // 4
            e = (j % 4) // 2
            jo = (j % 2) * P
            if (q, j) not in psum_tiles:
                psum_tiles[(q, j)] = mpsum.tile([P, 512], f32, tag="ps",
                                                name=f"ps_{q}_{j}")
            nc.tensor.matmul(
                psum_tiles[(q, j)],
                lhsT=xh[h][g][e][:, :, jo:jo + P],
                rhs=w8h[q][h],
                start=(h == 0), stop=(h == NH - 1),
                perf_mode=DR,
            )

        def evict(q, j, eng="act"):
            ps = psum_tiles.pop((q, j))
            sc = EV_SCALE_J[j]
            if eng == "act":
                nc.scalar.activation(ost[j][q], ps, COPY, scale=sc)
            else:
                nc.vector.tensor_scalar_mul(ost[j][q], ps, sc)

        def store(j, q, ring=None):
            (ring or nc.sync).dma_start(
                o_ap[j, :, 512 * q:512 * (q + 1)], ost[j][q])

        def evict_store(q, j, eng="act"):
            evict(q, j, eng)
            store(j, q, ring=nc.gpsimd)

        # ---------------- emission weave ----------------
        # Pass 0 m-group 0 (j0-3), t=0: halves already in flight.
        for k, xs_, ws_ in ((0, xsa, wsa), (2, xsb, wsb)):
            h = k // 2
            nc.vector.tensor_scalar(w8h[0][h], ws_[:, 0:2, :],
                                    128.0, 0.5, LT, SUB)
            nc.scalar.activation(xh[h][0][0], xs_[:, 0, :, :],
                                 SIGN, bias=127.5, scale=-1.0)
            nc.vector.tensor_scalar(xh[h][0][1], xs_[:, 1, :, :],
                                    128.0, 0.5, LT, SUB)
            for j in range(4):
                mm(0, j, h)
        for t in range(1, 4):
            xs = dma_x(t, 0, ring=nc.sync)
            ws = dma_w(0, t, ring=nc.sync)
            bin_x(t, 0, xs)
            bin_w(0, t, ws)
            for h in (2 * t, 2 * t + 1):
                for j in range(4):
                    mm(0, j, h)

        # Pass 0 m-group 1 (j4-7): x g1 granules first, then w q1
        # prefetch (not needed until pass 1, so it yields ring FIFO
        # priority to x during the bandwidth-critical pass-0 window);
        # evicts+stores of group 0 interleave at the end of the window.
        # The h=7 row is deferred: it fuses with the next group's h=0.
        for t in range(4):
            xs = dma_x(t, 1, ring=nc.sync)
            bin_x(t, 1, xs)
            for h in (2 * t, 2 * t + 1):
                if h == NH - 1:
                    continue  # deferred into the boundary fusion
                for j in range(4, MT):
                    mm(0, j, h)
            if t >= 2:
                evict_store(0, 2 * (t - 2), "dve")
                evict_store(0, 2 * (t - 2) + 1, "act")
        for t in range(4):
            ws = dma_w(1, t, ring=nc.sync)
            bin_w(1, t, ws)

        # passes 1..3: 4-wide groups; each group's h=0 row is fused
        # j-by-j with the previous group's deferred h=7 row, hiding the
        # chain start/stop pipeline bubble at every group boundary.
        # w q2/q3 arrive as single 1MB prefetches issued during pass 1.
        wbig_tiles = {}
        pending_h7 = [(0, j) for j in range(4, MT)]
        for q in range(1, NQ):
            for g in range(2):
                if q == NQ - 1 and g == 1:
                    break  # the final group is emitted below
                js = list(range(4 * g, 4 * g + 4))
                for (pq, pj), j in zip(pending_h7, js):
                    mm(pq, pj, NH - 1)
                    mm(q, j, 0)
                pend = [(qq, jj) for (qq, jj) in psum_tiles
                        if (qq, jj // 4) != (q, g)]
                ei = 0
                for h in range(1, NH - 1):
                    if q == 1 and g == 0 and h == 1:
                        wbig_tiles[2] = dma_w_pass(2, ring=nc.sync)
                    if q == 1 and g == 0 and h == 2:
                        wbig_tiles[3] = dma_w_pass(3, ring=nc.sync)
                    if q + 1 < NQ and g == 0 and 1 <= h <= 4:
                        for hh in (2 * (h - 1), 2 * (h - 1) + 1):
                            bin_w_big(q + 1, hh, wbig_tiles[q + 1])
                    for j in js:
                        mm(q, j, h)
                    if h < 5 and ei < len(pend):
                        evict_store(*pend[ei], "act")
                        ei += 1
                for tpl in pend[ei:]:
                    evict_store(*tpl, "act")
                pending_h7 = [(q, j) for j in js]

        # ---- final group (q=3, g=1): two ping-pong chain pairs with
        # inline evict+store; fuses the deferred h7 rows of (3, j0-3).
        q = NQ - 1
        pa = (4, 5)
        for (pq, pj), j in zip(pending_h7[0:2], pa):
            mm(pq, pj, NH - 1)
            mm(q, j, 0)
        for (pq, pj), j in zip(pending_h7[2:4], pa):
            mm(pq, pj, NH - 1)
            mm(q, j, 1)
        # evict+store the four fused chains immediately (alternating
        # engines and rings) so their store DMAs complete well before
        # the final stores land on the SDMA queues
        pend = [(qq, jj) for (qq, jj) in psum_tiles if jj < 4]
        for i, tpl in enumerate(pend):
            evict_store(*tpl, "act" if i % 2 else "dve")
        for h in range(2, NH):
            for j in pa:
                mm(q, j, h)
        # pair B serialized: j6's chain completes first and its
        # evict+store run during j7's rows, so the post-stream tail is
        # a single chain. j7's final evict is split across BOTH
        # elementwise engines into two half tiles, and its store goes
        # out as two 64KB DMAs on separate rings, shrinking the final
        # write-receipt window.
        ost7a = resident.tile([P, 256], f16, name="ost7a")
        ost7b = resident.tile([P, 256], f16, name="ost7b")
        for h in range(NH):
            mm(q, 6, h)
            if h == 1:
                evict(q, 4, "act")
                evict(q, 5, "dve")
            if h == 2:
                store(4, 3, ring=nc.gpsimd)
                store(5, 3, ring=nc.gpsimd)
        evict(q, 6, "act")
        store(6, 3, ring=nc.sync)
        for h in range(NH):
            mm(q, 7, h)
        ps7 = psum_tiles.pop((q, 7))
        nc.scalar.activation(ost7a, ps7[:, 0:256], COPY, scale=EV_SCALE_J[7])
        nc.vector.tensor_scalar_mul(ost7b, ps7[:, 256:512], EV_SCALE_J[7])
        nc.sync.dma_start(o_ap[7, :, 1536:1792], ost7a)
        nc.scalar.dma_start(o_ap[7, :, 1792:2048], ost7b)

    nc.compile()
    return nc


_NC_CACHE = {}
LAST_RESULTS = {}


def _get_nc():
    if "nc" not in _NC_CACHE:
        _NC_CACHE["nc"] = build_kernel()
    return _NC_CACHE["nc"]


def _prep_inputs(x, w):
    """Host-side formatting only: byte-plane slice + retile (no math)."""
    # high byte of each little-endian f32 = sign bit + exp[7:1]
    x_hi = x.view(np.uint8).reshape(B_FULL, D_IN, 4)[:, :, 3]
    w_hi = w.view(np.uint8).reshape(D_IN, UNITS, 4)[:, :, 3]
    # w: [d, u] -> [p, q, s, u']  with d = s*128 + p, u = q*512 + u'
    wt = w_hi.reshape(KT, P, NQ, 512).transpose(1, 2, 0, 3)
    w_core = np.ascontiguousarray(wt)
    in_maps = []
    for c in range(N_CORES):
        shard = x_hi[c * B_CORE:(c + 1) * B_CORE]          # [m, d]
        # [p, g, e, ks, m''] with m = g*512 + e*256 + m'', d = ks*128+p
        t = shard.T.reshape(KT, P, 2, 2, 256).transpose(1, 2, 3, 0, 4)
        in_maps.append({
            "xhi": np.ascontiguousarray(t),             # [128,2,2,16,256]
            "whi": w_core,
        })
    return in_maps


def kernel(x, w, _trace=False, _trace_cores=None):
    from concourse.bass_utils import run_bass_kernel_spmd

    x = np.asarray(x, dtype=np.float32)
    w = np.asarray(w, dtype=np.float32)
    assert x.shape == (B_FULL, D_IN) and w.shape == (D_IN, UNITS)

    nc = _get_nc()
    in_maps = _prep_inputs(x, w)
    br = run_bass_kernel_spmd(
        nc, in_maps, list(range(N_CORES)),
        trace=_trace, trace_cores=_trace_cores,
    )
    LAST_RESULTS["br"] = br
    out = np.concatenate(
        [br.results[c]["out"].astype(np.float32) for c in range(N_CORES)],
        axis=0,
    )
    return out


if __name__ == "__main__":
    rng = np.random.default_rng(0)
    x = rng.standard_normal((B_FULL, D_IN), dtype=np.float32)
    w = (rng.standard_normal((D_IN, UNITS), dtype=np.float32) * 0.1).astype(
        np.float32
    )
    out = kernel(x, w)
    exp = np.sign(x + (x == 0)) @ np.sign(w + (w == 0))
    print("max abs err:", np.max(np.abs(out - exp)))


# revision 31
# speedup vs baseline: 1.0019x; 1.0019x over previous
"""BinaryDense kernel for Trainium2: out = sign(x) @ sign(w).

x: [8192, 2048] f32, w: [2048, 2048] f32 -> out: [8192, 2048] f32.

Strategy: data-parallel shard of the batch dim across 8 NeuronCores
(1024 rows each, w replicated). The host ships only the HIGH BYTE of
each f32 (a pure byte-plane slice -- sign bit + 7 exponent bits, which
fully determines the sign), so input DMA is 6MB/core (x 2MB + w 4MB).
Host pre-tiles layouts so every DMA granule moves contiguous >=512B
lines per partition and every binarize op is fully contiguous:
  x_dram [128p, 2g, 2e, 16ks, 256m]  (d = ks*128 + p,
                                      m = g*512 + e*256 + m'')
  w_dram [128p, 4q, 16ks, 512u]      (u = q*512 + u')
Per core:
  - input DMAs are the first instructions on both HWDGE rings (x on
    sync, w on scalar), so data flows during the engine prologue;
  - PE warm-up dummy matmuls with NO dependencies (they read an
    uninitialized scratch tile; results are discarded) start the HAM
    clock ramp at the earliest point and bridge until real operands
    arrive, so the clock never re-gates;
  - binarized operands live in DEPENDENCY-EXACT tiles (the Tile
    framework tracks deps per tile): one [128,2,256] x-tile per
    (h-step, m-group, engine class) and one [128,2,512] w-tile per
    (pass, h-step), each written by exactly ONE binarize op, so every
    matmul waits on precisely the data it consumes;
  - binarize from the u8 high byte ((v < 128) <=> x >= 0), split
    across both elementwise engines by output-column class (the per-j
    evict scale absorbs the class difference):
      all w, and x e=1 cols (j2-3/j6-7) -> +-0.5 fp8e4 on DVE
      x e=0 cols (j0-1/j4-5)            -> +-1  fp8e4 on ACT
  - fp8 DoubleRow matmuls (K=256/instr, N=512): stationary = x slice,
    moving = w tile. Pass 0 runs h-major rows over m-group 0 then 1
    (8 PSUM banks = 8 concurrent chains), woven with the arrival
    stream; passes 1-3 run 4-wide j-groups with w prefetched a pass
    ahead (q2/q3 as single 1MB DMAs) and evicts of the previous group
    on the first rows. The final group runs as two ping-pong chain
    pairs with inline evict+store so the post-stream tail is one
    evict pair + two 128KB stores;
  - outputs in per-(j,q) fp16 tiles (integers <= 2048, exact); ring
    policy: ALL input DMAs issue on the sync ring (its queue is
    otherwise empty; the scalar ring's issue queue is shared with ACT
    compute and head-of-line blocks), mid-kernel stores go on the
    idle gpsimd SWDGE ring, and only the final two stores use the
    (by then empty) sync+scalar rings.

All arithmetic exact; host fp16->f32 widening exact.
"""

import sys

if "/opt/trn_rl_repo" not in sys.path:
    sys.path.insert(0, "/opt/trn_rl_repo")

import numpy as np

B_FULL, D_IN, UNITS = 8192, 2048, 2048
N_CORES = 8
B_CORE = B_FULL // N_CORES  # 1024
P = 128
KT = D_IN // P              # 16 k-subtiles
NQ = 4                      # n passes of 512 columns
NH = 8                      # DR h-steps per pass (2 k-subtiles each)
MT = B_CORE // P            # 8 m-tiles
N_DUMMY = 9                 # PE warm-up matmuls


def build_kernel():
    from concourse import bacc
    import concourse.mybir as mybir
    import concourse.tile as tile

    f32 = mybir.dt.float32
    f16 = mybir.dt.float16
    f8 = mybir.dt.float8e4
    u8 = mybir.dt.uint8

    LT = mybir.AluOpType.is_lt
    SUB = mybir.AluOpType.subtract
    COPY = mybir.ActivationFunctionType.Copy
    SIGN = mybir.ActivationFunctionType.Sign
    DR = mybir.MatmulPerfMode.DoubleRow

    # per-j evict scale: chain sum = alpha * out, w always +-0.5 (DVE)
    #   x on ACT (+-1, e=0) -> alpha 0.5 -> scale 2 : j0-1, j4-5
    #   x on DVE (+-0.5, e=1) -> alpha 0.25 -> scale 4 : j2-3, j6-7
    EV_SCALE_J = (2.0, 2.0, 4.0, 4.0, 2.0, 2.0, 4.0, 4.0)

    nc = bacc.Bacc("TRN2", target_bir_lowering=False)
    # const AP for the Sign-activation bias (127.5). The memset runs on
    # gpsimd immediately after the framework's init barrier; its first
    # reader (ACT Sign) is gated on input-DMA arrival, so no extra
    # barrier is needed.
    _bt = nc.alloc_sbuf_tensor("const-f32-127p5", [P, 1], f32)
    nc.gpsimd.memset(_bt.ap(), 127.5)
    nc.const_aps.aps[(f32, 127.5)] = _bt.ap()
    # scratch for the PE warm-up dummies: deliberately UNINITIALIZED.
    # The dummies only exist to keep the PE clock ramping; their
    # results go to a psum bank that is overwritten (start=True).
    _scratch = nc.alloc_sbuf_tensor("warm-scratch", [P, 2, 512], f8)

    x_d = nc.dram_tensor("xhi", [P, 2, 2, KT, 256], u8,
                         kind="ExternalInput")
    w_d = nc.dram_tensor("whi", [P, NQ, KT, 512], u8, kind="ExternalInput")
    o_d = nc.dram_tensor("out", [B_CORE, UNITS], f16, kind="ExternalOutput")

    o_ap = o_d[:].rearrange("(j p) u -> j p u", p=P)  # [MT, 128, U]

    with tile.TileContext(nc) as tc, \
         tc.tile_pool(name="wstage", bufs=10) as wstage, \
         tc.tile_pool(name="wbig", bufs=2) as wbig, \
         tc.tile_pool(name="xstage", bufs=9) as xstage, \
         tc.tile_pool(name="resident", bufs=1) as resident, \
         tc.tile_pool(name="mpsum", bufs=8, space="PSUM") as mpsum:

        # dependency-exact resident tiles, one writer each:
        # xh[h][g][e]: ksubs 2h..2h+1, m-cols g*512+e*256 .. +255
        xh = [[[resident.tile([P, 2, 256], f8, name=f"xh_{h}_{g}_{e}")
                for e in range(2)] for g in range(2)] for h in range(NH)]
        # w8h[q][h]: pass q, ksubs 2h..2h+1
        w8h = [[resident.tile([P, 2, 512], f8, name=f"w8_{q}_{h}")
                for h in range(NH)] for q in range(NQ)]
        # per-(j, q) output tiles: each written by ONE evict and read by
        # ONE store, so stores stream as soon as their chain evicts
        ost = [[resident.tile([P, 512], f16, name=f"ost_{j}_{q}")
                for q in range(NQ)] for j in range(MT)]

        # ---- head DMAs: first instructions on both HWDGE rings ----
        # t=0 granules split in k-halves; x halves on sync, w halves on
        # scalar, so the packet round-robin delivers the h=0 operand
        # pair (xsa+wsa) first, then the h=1 pair.
        xsa = xstage.tile([P, 2, 2, 256], u8, tag="xs", name="xs_0a")
        nc.sync.dma_start(xsa, x_d[:][:, 0, :, 0:2, :])
        wsa = wstage.tile([P, 2, 512], u8, tag="ws", name="ws_0a")
        nc.scalar.dma_start(wsa, w_d[:][:, 0, 0:2, :])
        xsb = xstage.tile([P, 2, 2, 256], u8, tag="xs", name="xs_0b")
        nc.sync.dma_start(xsb, x_d[:][:, 0, :, 2:4, :])
        wsb = wstage.tile([P, 2, 512], u8, tag="ws", name="ws_0b")
        nc.scalar.dma_start(wsb, w_d[:][:, 0, 2:4, :])

        # ---- PE warm-up: dummy matmuls with no dependencies ----
        ps_warm = mpsum.tile([P, 512], f32, tag="ps", name="ps_warm")
        for _ in range(N_DUMMY):
            nc.tensor.matmul(ps_warm, lhsT=_scratch.ap()[:, :, 0:P],
                             rhs=_scratch.ap(),
                             start=True, stop=True, perf_mode=DR)

        def dma_x(t, g, ring):
            xs = xstage.tile([P, 2, 4, 256], u8, tag="xs",
                             name=f"xs_{t}_{g}")
            ring.dma_start(xs, x_d[:][:, g, :, 4 * t:4 * t + 4, :])
            return xs

        def dma_w(q, hp, ring):
            ws = wstage.tile([P, 4, 512], u8, tag="ws", name=f"ws_{q}_{hp}")
            ring.dma_start(ws, w_d[:][:, q, 4 * hp:4 * hp + 4, :])
            return ws

        def dma_w_pass(q, ring):
            # whole 1MB pass prefetch in one DMA (8KB/partition lines)
            ws = wbig.tile([P, KT, 512], u8, tag="wb", name=f"wsbig_{q}")
            ring.dma_start(ws, w_d[:][:, q, :, :])
            return ws

        def bin_x(t, g, xs):
            # one contiguous op per (h, e): exact dep for its mms
            for c in (0, 2):
                h = 2 * t + c // 2
                nc.scalar.activation(xh[h][g][0], xs[:, 0, c:c + 2, :],
                                     SIGN, bias=127.5, scale=-1.0)
                nc.vector.tensor_scalar(xh[h][g][1], xs[:, 1, c:c + 2, :],
                                        128.0, 0.5, LT, SUB)

        def bin_w(q, hp, ws):
            for c in (0, 2):
                nc.vector.tensor_scalar(w8h[q][2 * hp + c // 2],
                                        ws[:, c:c + 2, :], 128.0, 0.5,
                                        LT, SUB)

        def bin_w_big(q, h, ws):
            nc.vector.tensor_scalar(w8h[q][h], ws[:, 2 * h:2 * h + 2, :],
                                    128.0, 0.5, LT, SUB)

        psum_tiles = {}

        def mm(q, j, h):
            g = j // 4
            e = (j % 4) // 2
            jo = (j % 2) * P
            if (q, j) not in psum_tiles:
                psum_tiles[(q, j)] = mpsum.tile([P, 512], f32, tag="ps",
                                                name=f"ps_{q}_{j}")
            nc.tensor.matmul(
                psum_tiles[(q, j)],
                lhsT=xh[h][g][e][:, :, jo:jo + P],
                rhs=w8h[q][h],
                start=(h == 0), stop=(h == NH - 1),
                perf_mode=DR,
            )

        def evict(q, j, eng="act"):
            ps = psum_tiles.pop((q, j))
            sc = EV_SCALE_J[j]
            if eng == "act":
                nc.scalar.activation(ost[j][q], ps, COPY, scale=sc)
            else:
                nc.vector.tensor_scalar_mul(ost[j][q], ps, sc)

        def store(j, q, ring=None):
            (ring or nc.sync).dma_start(
                o_ap[j, :, 512 * q:512 * (q + 1)], ost[j][q])

        def evict_store(q, j, eng="act"):
            evict(q, j, eng)
            store(j, q, ring=nc.gpsimd)

        # ---------------- emission weave ----------------
        # Pass 0 m-group 0 (j0-3), t=0: halves already in flight.
        for k, xs_, ws_ in ((0, xsa, wsa), (2, xsb, wsb)):
            h = k // 2
            nc.vector.tensor_scalar(w8h[0][h], ws_[:, 0:2, :],
                                    128.0, 0.5, LT, SUB)
            nc.scalar.activation(xh[h][0][0], xs_[:, 0, :, :],
                                 SIGN, bias=127.5, scale=-1.0)
            nc.vector.tensor_scalar(xh[h][0][1], xs_[:, 1, :, :],
                                    128.0, 0.5, LT, SUB)
            for j in range(4):
                mm(0, j, h)
        for t in range(1, 4):
            xs = dma_x(t, 0, ring=nc.sync)
            ws = dma_w(0, t, ring=nc.sync)
            bin_x(t, 0, xs)
            bin_w(0, t, ws)
            for h in (2 * t, 2 * t + 1):
                for j in range(4):
                    mm(0, j, h)

        # Pass 0 m-group 1 (j4-7): x g1 granules first, then w q1
        # prefetch (not needed until pass 1, so it yields ring FIFO
        # priority to x during the bandwidth-critical pass-0 window);
        # evicts+stores of group 0 interleave at the end of the window.
        # The h=7 row is deferred: it fuses with the next group's h=0.
        for t in range(4):
            xs = dma_x(t, 1, ring=nc.sync)
            bin_x(t, 1, xs)
            for h in (2 * t, 2 * t + 1):
                if h == NH - 1:
                    continue  # deferred into the boundary fusion
                for j in range(4, MT):
                    mm(0, j, h)
            if t >= 2:
                evict_store(0, 2 * (t - 2), "dve")
                evict_store(0, 2 * (t - 2) + 1, "act")
        for t in range(4):
            ws = dma_w(1, t, ring=nc.sync)
            bin_w(1, t, ws)

        # passes 1..3: 4-wide groups; each group's h=0 row is fused
        # j-by-j with the previous group's deferred h=7 row, hiding the
        # chain start/stop pipeline bubble at every group boundary.
        # w q2/q3 arrive as single 1MB prefetches issued during pass 1.
        wbig_tiles = {}
        pending_h7 = [(0, j) for j in range(4, MT)]
        for q in range(1, NQ):
            for g in range(2):
                if q == NQ - 1 and g == 1:
                    break  # the final group is emitted below
                js = list(range(4 * g, 4 * g + 4))
                for (pq, pj), j in zip(pending_h7, js):
                    mm(pq, pj, NH - 1)
                    mm(q, j, 0)
                pend = [(qq, jj) for (qq, jj) in psum_tiles
                        if (qq, jj // 4) != (q, g)]
                ei = 0
                for h in range(1, NH - 1):
                    if q == 1 and g == 0 and h == 1:
                        wbig_tiles[2] = dma_w_pass(2, ring=nc.sync)
                    if q == 1 and g == 0 and h == 2:
                        wbig_tiles[3] = dma_w_pass(3, ring=nc.sync)
                    if q + 1 < NQ and g == 0 and 1 <= h <= 4:
                        for hh in (2 * (h - 1), 2 * (h - 1) + 1):
                            bin_w_big(q + 1, hh, wbig_tiles[q + 1])
                    for j in js:
                        mm(q, j, h)
                    if h < 5 and ei < len(pend):
                        evict_store(*pend[ei], "act")
                        ei += 1
                for tpl in pend[ei:]:
                    evict_store(*tpl, "act")
                pending_h7 = [(q, j) for j in js]

        # ---- final group (q=3, g=1): two ping-pong chain pairs with
        # inline evict+store; fuses the deferred h7 rows of (3, j0-3).
        q = NQ - 1
        pa = (4, 5)
        for (pq, pj), j in zip(pending_h7[0:2], pa):
            mm(pq, pj, NH - 1)
            mm(q, j, 0)
        for (pq, pj), j in zip(pending_h7[2:4], pa):
            mm(pq, pj, NH - 1)
            mm(q, j, 1)
        # evict+store the four fused chains immediately (alternating
        # engines and rings) so their store DMAs complete well before
        # the final stores land on the SDMA queues
        pend = [(qq, jj) for (qq, jj) in psum_tiles if jj < 4]
        for i, tpl in enumerate(pend):
            evict_store(*tpl, "act" if i % 2 else "dve")
        for h in range(2, NH):
            for j in pa:
                mm(q, j, h)
        # pair B serialized: j6's chain completes first and its
        # evict+store run during j7's rows, so the post-stream tail is
        # a single chain. j7's final evict is split across BOTH
        # elementwise engines into two half tiles, and its store goes
        # out as two 64KB DMAs on separate rings, shrinking the final
        # write-receipt window.
        ost7a = resident.tile([P, 256], f16, name="ost7a")
        ost7b = resident.tile([P, 256], f16, name="ost7b")
        for h in range(NH):
            mm(q, 6, h)
            if h == 1:
                evict(q, 4, "act")
                evict(q, 5, "dve")
            if h == 2:
                store(4, 3, ring=nc.gpsimd)
                store(5, 3, ring=nc.gpsimd)
        evict(q, 6, "act")
        store(6, 3, ring=nc.sync)
        for h in range(NH):
            mm(q, 7, h)
        ps7 = psum_tiles.pop((q, 7))
        nc.scalar.activation(ost7a, ps7[:, 0:256], COPY, scale=EV_SCALE_J[7])
        nc.vector.tensor_scalar_mul(ost7b, ps7[:, 256:512], EV_SCALE_J[7])
        nc.sync.dma_start(o_ap[7, :, 1536:1792], ost7a)
        nc.scalar.dma_start(o_ap[7, :, 1792:2048], ost7b)

    nc.compile()
    return nc


_NC_CACHE = {}
LAST_RESULTS = {}


def _get_nc():
    if "nc" not in _NC_CACHE:
        _NC_CACHE["nc"] = build_kernel()
    return _NC_CACHE["nc"]


def _prep_inputs(x, w):
    """Host-side formatting only: byte-plane slice + retile (no math)."""
    # high byte of each little-endian f32 = sign bit + exp[7:1]
    x_hi = x.view(np.uint8).reshape(B_FULL, D_IN, 4)[:, :, 3]
    w_hi = w.view(np.uint8).reshape(D_IN, UNITS, 4)[:, :, 3]
    # w: [d, u] -> [p, q, s, u']  with d = s*128 + p, u = q*512 + u'
    wt = w_hi.reshape(KT, P, NQ, 512).transpose(1, 2, 0, 3)
    w_core = np.ascontiguousarray(wt)
    in_maps = []
    for c in range(N_CORES):
        shard = x_hi[c * B_CORE:(c + 1) * B_CORE]          # [m, d]
        # [p, g, e, ks, m''] with m = g*512 + e*256 + m'', d = ks*128+p
        t = shard.T.reshape(KT, P, 2, 2, 256).transpose(1, 2, 3, 0, 4)
        in_maps.append({
            "xhi": np.ascontiguousarray(t),             # [128,2,2,16,256]
            "whi": w_core,
        })
    return in_maps


def kernel(x, w, _trace=False, _trace_cores=None):
    from concourse.bass_utils import run_bass_kernel_spmd

    x = np.asarray(x, dtype=np.float32)
    w = np.asarray(w, dtype=np.float32)
    assert x.shape == (B_FULL, D_IN) and w.shape == (D_IN, UNITS)

    nc = _get_nc()
    in_maps = _prep_inputs(x, w)
    br = run_bass_kernel_spmd(
        nc, in_maps, list(range(N_CORES)),
        trace=_trace, trace_cores=_trace_cores,
    )
    LAST_RESULTS["br"] = br
    out = np.concatenate(
        [br.results[c]["out"].astype(np.float32) for c in range(N_CORES)],
        axis=0,
    )
    return out


if __name__ == "__main__":
    rng = np.random.default_rng(0)
    x = rng.standard_normal((B_FULL, D_IN), dtype=np.float32)
    w = (rng.standard_normal((D_IN, UNITS), dtype=np.float32) * 0.1).astype(
        np.float32
    )
    out = kernel(x, w)
    exp = np.sign(x + (x == 0)) @ np.sign(w + (w == 0))
    print("max abs err:", np.max(np.abs(out - exp)))
